# revision 5
# baseline (speedup 1.0000x reference)
"""GAT (3-layer, no-LeakyReLU) on 8 Trainium2 NeuronCores — v2.

Math: softmax is separable (no LeakyReLU): with aj[n,h] = <h[n,h,:],
att[h,C:]> and u = exp(aj),
    out[d] = sum_{e: dst=d} u[src_e]*h[src_e] / sum_e u[src_e].

v2 structure (vs v1 baseline):
  - Host LPT permutation balances edges per (core, dst-block) bin.
  - Layer 1 builds its gather table by REPLICATED dense (x is staged on
    every core) -> no AllGather and no cross-core sync until layer 2;
    startup skew is absorbed by layer-1 work.
  - NCH=2 source chunks: AllGather is split per chunk and overlaps the
    previous chunk's gather/aggregate pass. Cross-chunk partials
    accumulate in a bf16 SBUF acc.
  - 516-col row layout [uh0|uh1|u01 | uh2|uh3|u23]: two 258-col matmuls
    produce F and S1 together (no separate 4-col pS matmul). L3 uses a
    single 101-col matmul.
  - Tables store PACKED rows (516 / 101 cols); dma_gather uses
    elem_step < elem_size, reading garbage tails that are never used.
    AllGather bytes drop 20% (L1/2) / 60% (L3).
"""

import numpy as np
import ml_dtypes

N = 20000
E = 320000
NCORES = 8
NSH = 2500            # real nodes per core
NSHP = 2560           # padded to 20 x 128
P = 128
RT = NSHP // P        # row tiles / dst blocks per core = 20
NCH = 2               # source chunks
CHR = NSHP // NCH     # local rows per chunk = 1280
CBL = RT // NCH       # blocks per chunk = 10
GB = 8                # gather batch: tiles per dma_gather

# layer configs
# L1/L2: row = [uh0|uh1|u0u1 (258) | uh2|uh3|u2u3 (258)] = 516 packed
# L3:    row = [uh (100) | u (1)] = 101 packed
L1 = dict(H=4, C=128, KB=1, GE=640, RW=516)
L2 = dict(H=4, C=128, KB=4, GE=640, RW=516)
L3 = dict(H=1, C=100, KB=4, GE=256, RW=101)

BF16 = ml_dtypes.bfloat16


def _balance_nodes(edge_index):
    """LPT: assign nodes to (core, block) bins balancing in-edge counts.
    Returns node -> (core, block, pos) arrays."""
    import heapq

    indeg = np.bincount(edge_index[1], minlength=N)
    order = np.argsort(-indeg, kind="stable")
    nbins = NCORES * RT
    heap = [(0, 0, b) for b in range(nbins)]  # (edges, nnodes, bin)
    heapq.heapify(heap)
    node_bin = np.zeros(N, np.int64)
    node_pos = np.zeros(N, np.int64)
    for n in order:
        while True:
            e, c, b = heapq.heappop(heap)
            if c < P:
                break
        node_bin[n] = b
        node_pos[n] = c
        heapq.heappush(heap, (e + int(indeg[n]), c + 1, b))
    core = node_bin // RT
    blk = node_bin % RT
    return core, blk, node_pos


def _preprocess(edge_index):
    """Permute nodes, build per-core gather indices + one-hot tiles in
    (source-chunk, dst-block) pass order."""
    src = edge_index[0].astype(np.int64)
    dst = edge_index[1].astype(np.int64)

    ncore, nblk, npos = _balance_nodes(edge_index)
    # local row of node within its core (block-major), chunk & chunk row
    lrow = nblk * P + npos
    nchunk = nblk // CBL
    crow = ncore * CHR + (lrow - nchunk * CHR)   # chunk-local table row

    kd = ncore[dst]
    bd = nblk[dst]
    pd = npos[dst]
    cs = nchunk[src]
    rs = crow[src]

    order = np.lexsort((rs, bd, cs, kd))
    kd_s, bd_s, pd_s, cs_s, rs_s = (a[order] for a in (kd, bd, pd, cs, rs))

    cnt = np.zeros((NCORES, NCH, RT), np.int64)
    for k in range(NCORES):
        m = kd_s == k
        idx2 = cs_s[m] * RT + bd_s[m]
        cnt[k] = np.bincount(idx2, minlength=NCH * RT).reshape(NCH, RT)

    tb = np.ceil(cnt / P).astype(np.int64).max(axis=0)    # [NCH, RT]
    tch = tb.sum(axis=1)
    T = int(tb.sum())
    nb_ch = [(int(t) + GB - 1) // GB for t in tch]
    NB = int(sum(nb_ch))

    chunk_of_tile = []
    block_of_tile = []
    first = []
    last = []
    grp_off = np.zeros((NCH, RT), np.int64)
    acc = 0
    for c in range(NCH):
        for b in range(RT):
            grp_off[c, b] = acc
            for t in range(tb[c, b]):
                chunk_of_tile.append(c)
                block_of_tile.append(b)
                first.append(t == 0)
                last.append(t == tb[c, b] - 1)
            acc += tb[c, b]
    chunk_of_tile = np.array(chunk_of_tile)
    block_of_tile = np.array(block_of_tile)
    first = np.array(first)
    last = np.array(last)

    batches = []   # (chunk, tile_start, n_tiles)
    tile_base = np.concatenate([[0], np.cumsum(tch)])
    for c in range(NCH):
        t0 = int(tile_base[c])
        for g in range(nb_ch[c]):
            s0 = t0 + g * GB
            batches.append((c, s0, min(GB, t0 + int(tch[c]) - s0)))

    idxs_all = np.zeros((NCORES, T * P), np.int64)
    onehot_all = np.zeros((NCORES, T, P, P), BF16)
    for k in range(NCORES):
        m = kd_s == k
        bk, pk, ck, rk = bd_s[m], pd_s[m], cs_s[m], rs_s[m]
        off = np.concatenate([[0],
                              np.cumsum(cnt[k].reshape(-1))])
        for c in range(NCH):
            for b in range(RT):
                j = c * RT + b
                e0, e1 = off[j], off[j + 1]
                n_e = e1 - e0
                if n_e == 0:
                    continue
                slots = grp_off[c, b] * P + np.arange(n_e)
                idxs_all[k, slots] = rk[e0:e1]
                tt = slots // P
                ee = slots % P
                onehot_all[k, tt, ee, pk[e0:e1]] = 1.0

    # wrap indices per gather batch (16 partitions, replicated to 128)
    gbc = GB * P // 16      # idx cols per batch
    idx_wrapped = np.zeros((NCORES, 16, NB * gbc), np.int16)
    for g, (c, s0, nt) in enumerate(batches):
        i0 = s0 * P
        n_i = nt * P
        chunk = idxs_all[:, i0:i0 + n_i].astype(np.int16)
        idx_wrapped[:, :, g * gbc: g * gbc + n_i // 16] = (
            chunk.reshape(NCORES, n_i // 16, 16).transpose(0, 2, 1)
        )
    idx_rep = np.tile(idx_wrapped, (1, 8, 1))

    oh_b = np.zeros((NCORES, NB, P, GB * P), BF16)
    for g, (c, s0, nt) in enumerate(batches):
        chunk = onehot_all[:, s0:s0 + nt]
        oh_b[:, g, :, :nt * P] = chunk.transpose(0, 2, 1, 3).reshape(
            NCORES, P, nt * P)

    return dict(
        T=T, NB=NB, tb=tb, batches=batches,
        chunk_of_tile=chunk_of_tile, block_of_tile=block_of_tile,
        first=first, last=last,
        idxs=idx_rep, onehot=oh_b,
        ncore=ncore, nblk=nblk, npos=npos,
    )


def _build_program(ep):
    import concourse.bacc as bacc
    import concourse.mybir as mybir
    import concourse.tile as tile
    from concourse.masks import make_identity

    T, NB = ep["T"], ep["NB"]
    bot, first, last = ep["block_of_tile"], ep["first"], ep["last"]
    cot, tb, batches = ep["chunk_of_tile"], ep["tb"], ep["batches"]
    fgb = [min(c for c in range(NCH) if tb[c, b] > 0) for b in range(RT)]
    lgb = [max(c for c in range(NCH) if tb[c, b] > 0) for b in range(RT)]
    f32, bf16, i16 = mybir.dt.float32, mybir.dt.bfloat16, mybir.dt.int16
    gbc = GB * P // 16

    nc = bacc.Bacc("TRN2", target_bir_lowering=False, debug=False,
                   num_devices=NCORES, num_swdge_queues=4)

    # ---- I/O ----
    xT_own_in = nc.dram_tensor("xT_own", [P, NSHP], bf16, kind="ExternalInput")
    xT_full_in = nc.dram_tensor("xT_full", [P, NCH, NCORES * CHR], bf16,
                                kind="ExternalInput")
    w1_in = nc.dram_tensor("w1", [P, L1["RW"]], bf16, kind="ExternalInput")
    w2_in = nc.dram_tensor("w2", [P, 4, L2["RW"]], bf16, kind="ExternalInput")
    w3_in = nc.dram_tensor("w3", [P, 4, L3["RW"]], bf16, kind="ExternalInput")
    oh_in = nc.dram_tensor("onehot", [NB, P, GB * P], bf16,
                           kind="ExternalInput")
    idx_in = nc.dram_tensor("idxs", [P, NB * gbc], i16, kind="ExternalInput")
    out_d = nc.dram_tensor("out", [NSHP, 100], f32, kind="ExternalOutput")

    # ---- internal DRAM ----
    # table rows are GE wide (gather elem_step must be a multiple of
    # 128 elements); only the first RW cols are written/used
    t1 = [nc.dram_tensor(f"t1_{c}", [NCORES * CHR, L1["GE"]], bf16)
          for c in range(NCH)]
    ag2 = nc.dram_tensor("ag2", [NSHP, L2["GE"]], bf16)
    ag3 = nc.dram_tensor("ag3", [NSHP, L3["GE"]], bf16)
    t2 = [nc.dram_tensor(f"t2_{c}", [NCORES * CHR, L2["GE"]], bf16,
                         addr_space="Shared") for c in range(NCH)]
    t3 = [nc.dram_tensor(f"t3_{c}", [NCORES * CHR, L3["GE"]], bf16,
                         addr_space="Shared") for c in range(NCH)]
    tables = [t1, t2, t3]

    with tile.TileContext(nc, num_cores=NCORES) as tc:
        with (
            tc.tile_pool(name="const", bufs=1) as cp,
            tc.tile_pool(name="sb", bufs=2) as sb,
            tc.tile_pool(name="gat", bufs=7) as gp,
            tc.tile_pool(name="small", bufs=4) as sp,
            tc.tile_pool(name="psum", bufs=2, space="PSUM") as psA,
        ):
            ident = cp.tile([P, P], bf16, tag="ident")
            make_identity(nc, ident[:])
            idx_sb = cp.tile([P, NB * gbc], i16, tag="idx")
            nc.sync.dma_start(out=idx_sb[:], in_=idx_in[:, :])
            w_sb = []
            for i, (L, wt) in enumerate(((L1, w1_in), (L2, w2_in), (L3, w3_in))):
                w = cp.tile([P, L["KB"], L["RW"]], bf16, tag=f"w{i}",
                            name=f"w{i}")
                nc.sync.dma_start(out=w[:, 0, :] if i == 0 else w[:],
                                  in_=wt.ap())
                w_sb.append(w)
            xT_own = cp.tile([P, NSHP], bf16, tag="xT1")
            nc.sync.dma_start(out=xT_own[:], in_=xT_own_in.ap())
            xT2 = cp.tile([P, 4, NSHP], bf16, tag="xT2", name="xT2")
            xT3 = cp.tile([P, 4, NSHP], bf16, tag="xT3", name="xT3")
            xT_next = [xT2, xT3, None]

            def dense_tile(li, lhsT, uh_out, is_l3):
                """matmuls + exp + u*h for one 128-node row tile.
                uh_out: SBUF [P, RW] destination."""
                L = (L1, L2, L3)[li]
                KB = L["KB"]
                pA = psA.tile([P, 258], f32, tag="pA")
                pB = None if is_l3 else psA.tile([P, 258], f32, tag="pB")
                for kb in range(KB):
                    lt = lhsT(kb)
                    if is_l3:
                        nc.tensor.matmul(pA[:, 0:101], lt, w_sb[li][:, kb, :],
                                         start=(kb == 0), stop=(kb == KB - 1))
                    else:
                        nc.tensor.matmul(pA[:], lt, w_sb[li][:, kb, 0:258],
                                         start=(kb == 0), stop=(kb == KB - 1))
                        nc.tensor.matmul(pB[:], lt, w_sb[li][:, kb, 258:516],
                                         start=(kb == 0), stop=(kb == KB - 1))
                if is_l3:
                    u = sp.tile([P, 1], f32, tag="uA")
                    nc.scalar.activation(u[:], pA[:, 100:101],
                                         mybir.ActivationFunctionType.Exp)
                    nc.vector.tensor_scalar_mul(uh_out[:, 0:100],
                                                pA[:, 0:100], u[:, 0:1])
                    nc.vector.tensor_copy(out=uh_out[:, 100:101], in_=u[:])
                else:
                    uA = sp.tile([P, 2], f32, tag="uA")
                    uB = sp.tile([P, 2], f32, tag="uB")
                    nc.scalar.activation(uA[:], pA[:, 256:258],
                                         mybir.ActivationFunctionType.Exp)
                    nc.scalar.activation(uB[:], pB[:, 256:258],
                                         mybir.ActivationFunctionType.Exp)
                    nc.vector.tensor_scalar_mul(uh_out[:, 0:128],
                                                pA[:, 0:128], uA[:, 0:1])
                    nc.vector.tensor_scalar_mul(uh_out[:, 128:256],
                                                pA[:, 128:256], uA[:, 1:2])
                    nc.vector.tensor_copy(out=uh_out[:, 256:258], in_=uA[:])
                    nc.vector.tensor_scalar_mul(uh_out[:, 258:386],
                                                pB[:, 0:128], uB[:, 0:1])
                    nc.vector.tensor_scalar_mul(uh_out[:, 386:514],
                                                pB[:, 128:256], uB[:, 1:2])
                    nc.vector.tensor_copy(out=uh_out[:, 514:516], in_=uB[:])

            for li, L in enumerate((L1, L2, L3)):
                H, C, KB, GE, RW = L["H"], L["C"], L["KB"], L["GE"], L["RW"]
                is_l3 = li == 2
                G_sb = sb.tile([P, RT, RW], bf16, tag="G", bufs=1)
                acc = sb.tile([P, RT, RW], bf16, tag="acc", bufs=1,
                              name="acc")

                # ---------- build table ----------
                if li == 0:
                    # replicated dense over ALL cores' rows, chunk-major
                    # (emitted before the local dense so pass-0 gathers
                    # unblock as early as possible)
                    for c in range(NCH):
                        for kk in range(NCORES):
                            xs = sb.tile([P, CHR], bf16, tag="xs", bufs=3)
                            nc.sync.dma_start(
                                out=xs[:],
                                in_=xT_full_in[:, c, kk * CHR:(kk + 1) * CHR])
                            for rr in range(CBL):
                                st = sb.tile([P, RW], bf16, tag="st", bufs=4)
                                dense_tile(
                                    0, lambda kb, rr=rr:
                                    xs[:, rr * P:(rr + 1) * P],
                                    st[:], False)
                                row0 = kk * CHR + rr * P
                                nc.sync.dma_start(
                                    out=t1[c][row0:row0 + P, 0:RW],
                                    in_=st[:])

                # ---------- local dense -> G_sb (self loops) ----------
                for r in range(RT):
                    if li == 0:
                        lh = lambda kb, r=r: xT_own[:, r * P:(r + 1) * P]
                    else:
                        lh = lambda kb, r=r: xT_next[li - 1][
                            :, kb, r * P:(r + 1) * P]
                    dense_tile(li, lh, G_sb[:, r, :], is_l3)
                    if li > 0:
                        # stream rows to the AllGather input
                        ag = ag2 if li == 1 else ag3
                        nc.sync.dma_start(
                            out=ag[r * P:(r + 1) * P, 0:RW],
                            in_=G_sb[:, r, :])

                if li > 0:
                    ag = ag2 if li == 1 else ag3
                    tbl = tables[li]
                    for c in range(NCH):
                        nc.gpsimd.collective_compute(
                            "AllGather", mybir.AluOpType.bypass,
                            replica_groups=[list(range(NCORES))],
                            ins=[ag[c * CHR:(c + 1) * CHR, :].opt()],
                            outs=[tbl[c][0:NCORES * CHR, :].opt()])

                # ---------- edge aggregation ----------
                pA = pB = None
                for g, (c, s0, nt) in enumerate(batches):
                    gt = gp.tile([P, GB, GE], bf16, tag="gt")
                    nc.gpsimd.dma_gather(
                        gt[:, :nt, :], tables[li][c].ap(),
                        idx_sb[:, g * gbc: g * gbc + nt * P // 16],
                        nt * P, nt * P, GE, queue_num=g % 4)
                    oh = gp.tile([P, GB, P], bf16, tag="oh", bufs=8)
                    nc.sync.dma_start(
                        out=oh[:, :nt, :],
                        in_=oh_in[g, :, :nt * P].rearrange(
                            "p (t d) -> p t d", d=P))
                    for tl in range(nt):
                        ti = s0 + tl
                        b = bot[ti]
                        if first[ti]:
                            pA = psA.tile([P, 258], f32, tag="pA")
                            if not is_l3:
                                pB = psA.tile([P, 258], f32, tag="pB")
                        if is_l3:
                            nc.tensor.matmul(pA[:, 0:101], oh[:, tl, :],
                                             gt[:, tl, 0:101],
                                             start=bool(first[ti]),
                                             stop=bool(last[ti]))
                        else:
                            nc.tensor.matmul(pA[:], oh[:, tl, :],
                                             gt[:, tl, 0:258],
                                             start=bool(first[ti]),
                                             stop=bool(last[ti]))
                            nc.tensor.matmul(pB[:], oh[:, tl, :],
                                             gt[:, tl, 258:516],
                                             start=bool(first[ti]),
                                             stop=bool(last[ti]))
                        if not last[ti]:
                            continue
                        # fold chunk partial into acc
                        aslc = acc[:, b, 0:RW]
                        if cot[ti] == fgb[b]:
                            if is_l3:
                                nc.vector.tensor_copy(out=aslc,
                                                      in_=pA[:, 0:101])
                            else:
                                nc.vector.tensor_copy(out=acc[:, b, 0:258],
                                                      in_=pA[:])
                                nc.vector.tensor_copy(out=acc[:, b, 258:516],
                                                      in_=pB[:])
                        else:
                            if is_l3:
                                nc.vector.tensor_add(out=aslc, in0=aslc,
                                                     in1=pA[:, 0:101])
                            else:
                                nc.vector.tensor_add(
                                    out=acc[:, b, 0:258],
                                    in0=acc[:, b, 0:258], in1=pA[:])
                                nc.vector.tensor_add(
                                    out=acc[:, b, 258:516],
                                    in0=acc[:, b, 258:516], in1=pB[:])
                        if cot[ti] != lgb[b]:
                            continue
                        # ---------- block evacuation ----------
                        ev = sb.tile([P, RW], bf16, tag="ev", bufs=2)
                        nc.vector.tensor_add(out=ev[:], in0=aslc,
                                             in1=G_sb[:, b, 0:RW])
                        if is_l3:
                            s1 = sp.tile([P, 1], f32, tag="s1")
                            nc.vector.tensor_scalar_max(
                                s1[:], ev[:, 100:101], 1e-30)
                            rec = sp.tile([P, 1], f32, tag="rec")
                            nc.vector.reciprocal(rec[:], s1[:])
                            o3 = sb.tile([P, 100], f32, tag="o3")
                            nc.vector.tensor_scalar_mul(
                                o3[:], ev[:, 0:100], rec[:, 0:1])
                            nc.sync.dma_start(
                                out=out_d[b * P:(b + 1) * P, :], in_=o3[:])
                        else:
                            s1 = sp.tile([P, 4], f32, tag="s1")
                            nc.vector.tensor_copy(out=s1[:, 0:2],
                                                  in_=ev[:, 256:258])
                            nc.vector.tensor_copy(out=s1[:, 2:4],
                                                  in_=ev[:, 514:516])
                            nc.vector.tensor_scalar_max(s1[:], s1[:], 1e-30)
                            rec = sp.tile([P, 4], f32, tag="rec")
                            nc.vector.reciprocal(rec[:], s1[:])
                            ob = sb.tile([P, 512], bf16, tag="ob")
                            hs = (0, 128, 258, 386)
                            for h in range(4):
                                nc.vector.tensor_scalar_mul(
                                    ob[:, h * 128:(h + 1) * 128],
                                    ev[:, hs[h]:hs[h] + 128],
                                    rec[:, h:h + 1])
                            for fb in range(4):
                                pt = psA.tile([P, P], bf16, tag="pt", bufs=1)
                                nc.tensor.transpose(
                                    pt[:], ob[:, fb * P:(fb + 1) * P],
                                    ident[:])
                                nc.vector.tensor_copy(
                                    out=xT_next[li][:, fb,
                                                    b * P:(b + 1) * P],
                                    in_=pt[:])
    nc.compile()
    return nc


def _prep_weights(W1, att1, W2, att2, W3, att3):
    """Fold att source-halves into per-kb [A|B] 516-col (or 101) layouts."""
    def fold(W, att, H, C, KB):
        # W: [H*C, F_in]; returns [F_in split into KB x 128, layout]
        F_in = W.shape[1]
        wt = W.T.reshape(KB, P, H * C)                    # [kb, f, out]
        wj = np.stack([att[h, C:] @ W[h * C:(h + 1) * C, :]
                       for h in range(H)], axis=1)        # [F_in, H]
        wj = wj.reshape(KB, P, H)
        if H == 4:
            out = np.zeros((KB, P, 516), np.float32)
            out[:, :, 0:256] = wt[:, :, 0:256]
            out[:, :, 256:258] = wj[:, :, 0:2]
            out[:, :, 258:514] = wt[:, :, 256:512]
            out[:, :, 514:516] = wj[:, :, 2:4]
        else:
            out = np.zeros((KB, P, 101), np.float32)
            out[:, :, 0:100] = wt[:, :, 0:100]
            out[:, :, 100:101] = wj
        return np.ascontiguousarray(out.transpose(1, 0, 2)).astype(BF16)

    return {
        "w1": fold(W1, att1, 4, 128, 1)[:, 0, :],       # [P, 516]
        "w2": fold(W2, att2, 4, 128, 4),                # [P, 4, 516]
        "w3": fold(W3, att3, 1, 100, 4),                # [P, 4, 101]
    }


_CACHE = {}


def _run(inputs, trace):
    from concourse.bass_utils import run_bass_kernel_spmd

    x = np.asarray(inputs["x"], np.float32)
    edge_index = np.asarray(inputs["edge_index"]).astype(np.int64)

    ep = _preprocess(edge_index)
    wd = _prep_weights(*[np.asarray(inputs[k], np.float32) for k in
                         ("W1", "att1", "W2", "att2", "W3", "att3")])

    ncore, nblk, npos = ep["ncore"], ep["nblk"], ep["npos"]
    # x in slot layout
    xb = x.astype(BF16)
    x_slot = np.zeros((NCORES, NSHP, P), BF16)           # [core, lrow, feat]
    lrow = nblk * P + npos
    x_slot[ncore, lrow] = xb
    # xT_full: [feat, chunk, core*CHR + chunkrow] (same for all cores)
    xt_full = np.zeros((P, NCH, NCORES * CHR), BF16)
    for c in range(NCH):
        seg = x_slot[:, c * CHR:(c + 1) * CHR]           # [core, CHR, feat]
        xt_full[:, c, :] = seg.reshape(NCORES * CHR, P).T

    key = ("prog", ep["T"], ep["NB"], ep["tb"].tobytes())
    if key not in _CACHE:
        _CACHE[key] = _build_program(ep)
    nc = _CACHE[key]

    in_maps = []
    for k in range(NCORES):
        m = dict(wd)
        m["xT_own"] = np.ascontiguousarray(x_slot[k].T)
        m["xT_full"] = xt_full
        m["onehot"] = ep["onehot"][k]
        m["idxs"] = ep["idxs"][k]
        in_maps.append(m)

    res = run_bass_kernel_spmd(nc, in_maps, core_ids=list(range(NCORES)),
                               trace=trace)
    out_full = np.stack([res.results[k]["out"] for k in range(NCORES)])
    out = np.zeros((N, 100), np.float32)
    out[np.arange(N)] = out_full[ncore, lrow]
    return out, res


def kernel(x, W1, att1, W2, att2, W3, att3, edge_index):
    out, _ = _run(dict(x=x, W1=W1, att1=att1, W2=W2, att2=att2, W3=W3,
                       att3=att3, edge_index=edge_index), trace=False)
    return out


def kernel_traced(inputs):
    return _run(inputs, trace=True)


# revision 6
# speedup vs baseline: 1.3873x; 1.3873x over previous
"""GAT (3-layer, no-LeakyReLU) on 8 Trainium2 NeuronCores — v3.

Math: softmax is separable (no LeakyReLU): with aj[n,h] = <h[n,h,:],
att[h,C:]> and u = exp(aj),
    out[d] = sum_{e: dst=d} u[src_e]*h[src_e] / sum_e u[src_e]
(the ai[dst] term cancels inside the per-destination softmax).

Per layer, per core (nodes sharded by destination):
  1. dense:  h = x @ W.T; u = exp(x @ wj); G row = [u*h | u] (640-wide)
  2. AllGather G -> replicated table in (pair-shared) HBM
  3. per dst-block of 128 nodes: dma_gather G[src] rows, one-hot matmul
     accumulates F = oh.T @ u*h and S1 = oh.T @ u
  4. out = F / S1; transpose to xT for the next layer's dense.

v3 over the v1 baseline:
  - Host LPT permutation balances in-edges per (core, dst-block) bin:
    tiles per layer drop 340 -> 320 and skew padding disappears.
  - L3 computes F and S1 in ONE 101-col matmul (row = [u*h | u]).
  - DVE ops fused: u*h and the output normalization use a single
    broadcast tensor_mul over [P,4,128] views; transposed blocks land in
    one [P,512] PSUM tile copied with one op.
  - Deeper gather pipeline (gt bufs 8).
"""

import numpy as np
import ml_dtypes

N = 20000
E = 320000
NCORES = 8
NSH = 2500            # real nodes per core
NSHP = 2560           # padded to 20 x 128
P = 128
RT = NSHP // P        # row tiles / dst blocks per core = 20
GB = 8                # gather batch: tiles per dma_gather

# layer configs: row = [u*h (H*C) | u (H)] padded to GW
L1 = dict(H=4, C=128, KB=1, GW=640, UO=512)
L2 = dict(H=4, C=128, KB=4, GW=640, UO=512)
L3 = dict(H=1, C=100, KB=4, GW=256, UO=100)

BF16 = ml_dtypes.bfloat16


def _balance_nodes(edge_index):
    """LPT: assign nodes to (core, block) bins balancing in-edge counts."""
    import heapq

    indeg = np.bincount(edge_index[1], minlength=N)
    order = np.argsort(-indeg, kind="stable")
    heap = [(0, 0, b) for b in range(NCORES * RT)]  # (edges, nnodes, bin)
    heapq.heapify(heap)
    node_bin = np.zeros(N, np.int64)
    node_pos = np.zeros(N, np.int64)
    for n in order:
        while True:
            e, c, b = heapq.heappop(heap)
            if c < P:
                break
        node_bin[n] = b
        node_pos[n] = c
        heapq.heappush(heap, (e + int(indeg[n]), c + 1, b))
    return node_bin // RT, node_bin % RT, node_pos


def _preprocess(edge_index):
    """Permute nodes; per-core gather indices + one-hot tiles grouped by
    dst block."""
    ncore, nblk, npos = _balance_nodes(edge_index)
    lrow = nblk * P + npos                       # local row within core

    src = edge_index[0].astype(np.int64)
    dst = edge_index[1].astype(np.int64)
    kd = ncore[dst]
    bd = nblk[dst]
    pd = npos[dst]
    rs = ncore[src] * NSHP + lrow[src]           # table row of source

    order = np.lexsort((rs, bd, kd))
    kd_s, bd_s, pd_s, rs_s = (a[order] for a in (kd, bd, pd, rs))

    cnt = np.zeros((NCORES, RT), np.int64)
    for k in range(NCORES):
        cnt[k] = np.bincount(bd_s[kd_s == k], minlength=RT)

    tb = np.ceil(cnt / P).astype(np.int64).max(axis=0)    # [RT]
    T = int(tb.sum())
    NB = (T + GB - 1) // GB

    block_of_tile = []
    first = []
    last = []
    grp_off = np.zeros(RT, np.int64)
    acc = 0
    for b in range(RT):
        grp_off[b] = acc
        for t in range(tb[b]):
            block_of_tile.append(b)
            first.append(t == 0)
            last.append(t == tb[b] - 1)
        acc += tb[b]
    block_of_tile = np.array(block_of_tile)
    first = np.array(first)
    last = np.array(last)

    idxs_all = np.zeros((NCORES, T * P), np.int64)
    onehot_all = np.zeros((NCORES, T, P, P), BF16)
    for k in range(NCORES):
        m = kd_s == k
        bk, pk, rk = bd_s[m], pd_s[m], rs_s[m]
        off = np.concatenate([[0], np.cumsum(cnt[k])])
        for b in range(RT):
            e0, e1 = off[b], off[b + 1]
            n_e = e1 - e0
            if n_e == 0:
                continue
            slots = grp_off[b] * P + np.arange(n_e)
            idxs_all[k, slots] = rk[e0:e1]
            onehot_all[k, slots // P, slots % P, pk[e0:e1]] = 1.0

    gbc = GB * P // 16
    idx_wrapped = np.zeros((NCORES, 16, NB * gbc), np.int16)
    for g in range(NB):
        i0 = g * GB * P
        n_i = min(GB * P, T * P - i0)
        chunk = idxs_all[:, i0:i0 + n_i].astype(np.int16)
        idx_wrapped[:, :, g * gbc: g * gbc + n_i // 16] = (
            chunk.reshape(NCORES, n_i // 16, 16).transpose(0, 2, 1)
        )
    idx_rep = np.tile(idx_wrapped, (1, 8, 1))

    oh_b = np.zeros((NCORES, NB, P, GB * P), BF16)
    for g in range(NB):
        nt = min(GB, T - g * GB)
        chunk = onehot_all[:, g * GB:g * GB + nt]
        oh_b[:, g, :, :nt * P] = chunk.transpose(0, 2, 1, 3).reshape(
            NCORES, P, nt * P)

    return dict(
        T=T, NB=NB, tb=tb,
        block_of_tile=block_of_tile, first=first, last=last,
        idxs=idx_rep, onehot=oh_b,
        ncore=ncore, nblk=nblk, npos=npos,
    )


def _build_program(ep):
    import concourse.bacc as bacc
    import concourse.mybir as mybir
    import concourse.tile as tile
    from concourse.masks import make_identity

    T, NB = ep["T"], ep["NB"]
    bot, first, last = ep["block_of_tile"], ep["first"], ep["last"]
    f32, bf16, i16 = mybir.dt.float32, mybir.dt.bfloat16, mybir.dt.int16
    gbc = GB * P // 16

    nc = bacc.Bacc("TRN2", target_bir_lowering=False, debug=False,
                   num_devices=NCORES, num_swdge_queues=4)

    # ---- I/O ----
    xT_in = nc.dram_tensor("xT", [P, NSHP], bf16, kind="ExternalInput")
    w1_in = nc.dram_tensor("w1", [P, 512], bf16, kind="ExternalInput")
    wj1_in = nc.dram_tensor("wj1", [P, 4], bf16, kind="ExternalInput")
    w2_in = nc.dram_tensor("w2", [P, 4, 512], bf16, kind="ExternalInput")
    wj2_in = nc.dram_tensor("wj2", [P, 4, 4], bf16, kind="ExternalInput")
    w3_in = nc.dram_tensor("w3", [P, 4, 101], bf16, kind="ExternalInput")
    oh_in = nc.dram_tensor("onehot", [NB, P, GB * P], bf16,
                           kind="ExternalInput")
    idx_in = nc.dram_tensor("idxs", [P, NB * gbc], i16, kind="ExternalInput")
    out_d = nc.dram_tensor("out", [NSHP, 100], f32, kind="ExternalOutput")

    # ---- internal DRAM ----
    ag = [nc.dram_tensor(f"ag{i}", [NSHP, L["GW"]], bf16)
          for i, L in enumerate((L1, L2, L3))]
    table = [nc.dram_tensor(f"table{i}", [NCORES * NSHP, L["GW"]], bf16,
                            addr_space="Shared")
             for i, L in enumerate((L1, L2, L3))]

    with tile.TileContext(nc, num_cores=NCORES) as tc:
        with (
            tc.tile_pool(name="const", bufs=1) as cp,
            tc.tile_pool(name="sb", bufs=2) as sb,
            tc.tile_pool(name="gat", bufs=8) as gp,
            tc.tile_pool(name="small", bufs=4) as sp,
            tc.tile_pool(name="psum", bufs=2, space="PSUM") as psA,
        ):
            ident = cp.tile([P, P], bf16, tag="ident")
            make_identity(nc, ident[:])
            idx_sb = cp.tile([P, NB * gbc], i16, tag="idx")
            nc.sync.dma_start(out=idx_sb[:], in_=idx_in[:, :])
            w_sb = []
            wj_sb = []
            for i, (L, wt, wj) in enumerate(
                ((L1, w1_in, wj1_in), (L2, w2_in, wj2_in), (L3, w3_in, None))
            ):
                nout = 512 if i < 2 else 101
                w = cp.tile([P, L["KB"], nout], bf16, tag=f"w{i}", name=f"w{i}")
                nc.sync.dma_start(out=w[:, 0, :] if i == 0 else w[:],
                                  in_=wt.ap())
                w_sb.append(w)
                if wj is not None:
                    wjt = cp.tile([P, L["KB"], L["H"]], bf16, tag=f"wj{i}",
                                  name=f"wj{i}")
                    nc.sync.dma_start(out=wjt[:, 0, :] if i == 0 else wjt[:],
                                      in_=wj.ap())
                    wj_sb.append(wjt)
                else:
                    wj_sb.append(None)

            xT1 = cp.tile([P, 1, NSHP], bf16, tag="xT1")
            nc.sync.dma_start(out=xT1[:, 0, :], in_=xT_in.ap())
            xT2 = cp.tile([P, 4, NSHP], bf16, tag="xT2", name="xT2")
            xT3 = cp.tile([P, 4, NSHP], bf16, tag="xT3", name="xT3")
            xT_next = [None, xT2, xT3]

            for li, L in enumerate((L1, L2, L3)):
                H, C, KB, GW, UO = L["H"], L["C"], L["KB"], L["GW"], L["UO"]
                is_l3 = li == 2
                G_sb = sb.tile([P, RT, GW], bf16, tag="G", bufs=1)
                xT = xT1 if li == 0 else xT_next[li]

                # ---------- dense + u ----------
                for r in range(RT):
                    if is_l3:
                        # single matmul: w3 holds [W | wj] = 101 cols
                        ph = psA.tile([P, 101], f32, tag="ph")
                        for kb in range(KB):
                            nc.tensor.matmul(ph[:], xT[:, kb, r * P:(r + 1) * P],
                                             w_sb[li][:, kb, :],
                                             start=(kb == 0),
                                             stop=(kb == KB - 1))
                        u = sp.tile([P, 1], f32, tag="u")
                        nc.scalar.activation(u[:], ph[:, 100:101],
                                             mybir.ActivationFunctionType.Exp)
                        nc.vector.tensor_scalar_mul(
                            G_sb[:, r, 0:100], ph[:, 0:100], u[:, 0:1])
                        nc.vector.tensor_copy(out=G_sb[:, r, 100:101],
                                              in_=u[:])
                    else:
                        ph = psA.tile([P, 512], f32, tag="ph")
                        pa = psA.tile([P, H], f32, tag="pa", bufs=1)
                        for kb in range(KB):
                            lhsT = xT[:, kb, r * P:(r + 1) * P]
                            nc.tensor.matmul(ph[:], lhsT, w_sb[li][:, kb, :],
                                             start=(kb == 0),
                                             stop=(kb == KB - 1))
                            nc.tensor.matmul(pa[:], lhsT, wj_sb[li][:, kb, :],
                                             start=(kb == 0),
                                             stop=(kb == KB - 1))
                        u = sp.tile([P, H], f32, tag="u")
                        nc.scalar.activation(u[:], pa[:],
                                             mybir.ActivationFunctionType.Exp)
                        # u*h in ONE broadcast multiply over [P,H,C]
                        nc.vector.tensor_mul(
                            out=G_sb[:, r, 0:512].rearrange(
                                "p (h c) -> p h c", c=C),
                            in0=ph[:].rearrange("p (h c) -> p h c", c=C),
                            in1=u[:].unsqueeze(2).to_broadcast([P, H, C]))
                        nc.vector.tensor_copy(out=G_sb[:, r, UO:UO + H],
                                              in_=u[:])
                    nc.sync.dma_start(
                        out=ag[li][r * P:(r + 1) * P, :],
                        in_=G_sb[:, r, :])

                # ---------- exchange ----------
                nc.gpsimd.collective_compute(
                    "AllGather", mybir.AluOpType.bypass,
                    replica_groups=[list(range(NCORES))],
                    ins=[ag[li].ap().opt()],
                    outs=[table[li].ap().opt()])

                # ---------- edge aggregation ----------
                pF = pS = None
                for g in range(NB):
                    s0 = g * GB
                    nt = min(GB, T - s0)
                    gt = gp.tile([P, GB, GW], bf16, tag="gt")
                    nc.gpsimd.dma_gather(
                        gt[:, :nt, :], table[li].ap(),
                        idx_sb[:, g * gbc: g * gbc + nt * P // 16],
                        nt * P, nt * P, GW, queue_num=g % 4)
                    oh = gp.tile([P, GB, P], bf16, tag="oh", bufs=10)
                    nc.sync.dma_start(
                        out=oh[:, :nt, :],
                        in_=oh_in[g, :, :nt * P].rearrange(
                            "p (t d) -> p t d", d=P))
                    for tl in range(nt):
                        ti = s0 + tl
                        b = bot[ti]
                        if first[ti]:
                            pF = psA.tile([P, 101 if is_l3 else 512], f32,
                                          tag="pF")
                            if not is_l3:
                                pS = psA.tile([P, H], f32, tag="pS")
                        if is_l3:
                            nc.tensor.matmul(pF[:], oh[:, tl, :],
                                             gt[:, tl, 0:101],
                                             start=bool(first[ti]),
                                             stop=bool(last[ti]))
                        else:
                            nc.tensor.matmul(pF[:], oh[:, tl, :],
                                             gt[:, tl, 0:512],
                                             start=bool(first[ti]),
                                             stop=bool(last[ti]))
                            nc.tensor.matmul(pS[:], oh[:, tl, :],
                                             gt[:, tl, UO:UO + H],
                                             start=bool(first[ti]),
                                             stop=bool(last[ti]))
                        if not last[ti]:
                            continue
                        # ---------- block evacuation ----------
                        if is_l3:
                            nc.vector.tensor_add(
                                out=pF[:], in0=pF[:],
                                in1=G_sb[:, b, 0:101])          # self loop
                            s1c = sp.tile([P, 1], f32, tag="s1c")
                            nc.vector.tensor_scalar_max(
                                s1c[:], pF[:, 100:101], 1e-30)
                            rec = sp.tile([P, 1], f32, tag="rec")
                            nc.vector.reciprocal(rec[:], s1c[:])
                            o3 = sb.tile([P, 100], f32, tag="o3")
                            nc.vector.tensor_scalar_mul(
                                o3[:], pF[:, 0:100], rec[:, 0:1])
                            nc.sync.dma_start(
                                out=out_d[b * P:(b + 1) * P, :], in_=o3[:])
                        else:
                            nc.vector.tensor_add(
                                out=pF[:], in0=pF[:],
                                in1=G_sb[:, b, 0:512])          # self loop
                            nc.vector.tensor_add(
                                out=pS[:], in0=pS[:],
                                in1=G_sb[:, b, UO:UO + H])
                            s1c = sp.tile([P, H], f32, tag="s1c")
                            nc.vector.tensor_scalar_max(s1c[:], pS[:], 1e-30)
                            rec = sp.tile([P, H], f32, tag="rec")
                            nc.vector.reciprocal(rec[:], s1c[:])
                            ob = sb.tile([P, 512], bf16, tag="ob")
                            nc.vector.tensor_mul(
                                out=ob[:].rearrange("p (h c) -> p h c", c=C),
                                in0=pF[:].rearrange("p (h c) -> p h c", c=C),
                                in1=rec[:].unsqueeze(2).to_broadcast(
                                    [P, H, C]))
                            pt = psA.tile([P, 4, P], bf16, tag="pt", bufs=1)
                            for fb in range(4):
                                nc.tensor.transpose(
                                    pt[:, fb, :], ob[:, fb * P:(fb + 1) * P],
                                    ident[:])
                            nc.vector.tensor_copy(
                                out=xT_next[li + 1][:, :, b * P:(b + 1) * P],
                                in_=pt[:])
    nc.compile()
    return nc


def _prep_weights(W1, att1, W2, att2, W3, att3):
    """Host-side weight folding and layout prep (fp32 -> bf16)."""
    def fold_wj(W, att, H, C):
        return np.stack([att[h, C:] @ W[h * C:(h + 1) * C, :] for h in range(H)],
                        axis=1)  # [F_in, H]

    d = {}
    d["w1"] = np.ascontiguousarray(W1.T).astype(BF16)                # [128, 512]
    d["wj1"] = fold_wj(W1, att1, 4, 128).astype(BF16)                # [128, 4]
    d["w2"] = np.ascontiguousarray(W2.T).reshape(4, 128, 512).transpose(
        1, 0, 2).copy().astype(BF16)                                  # [128,4,512]
    d["wj2"] = fold_wj(W2, att2, 4, 128).reshape(4, 128, 4).transpose(
        1, 0, 2).copy().astype(BF16)                                  # [128,4,4]
    w3 = np.ascontiguousarray(W3.T).reshape(4, 128, 100)              # [4,128,100]
    wj3 = fold_wj(W3, att3, 1, 100).reshape(4, 128, 1)                # [4,128,1]
    d["w3"] = np.concatenate([w3, wj3], axis=2).transpose(
        1, 0, 2).copy().astype(BF16)                                  # [128,4,101]
    return d


_CACHE = {}


def _run(inputs, trace):
    from concourse.bass_utils import run_bass_kernel_spmd

    x = np.asarray(inputs["x"], np.float32)
    edge_index = np.asarray(inputs["edge_index"]).astype(np.int64)

    ep = _preprocess(edge_index)
    wd = _prep_weights(*[np.asarray(inputs[k], np.float32) for k in
                         ("W1", "att1", "W2", "att2", "W3", "att3")])

    ncore, nblk, npos = ep["ncore"], ep["nblk"], ep["npos"]
    lrow = nblk * P + npos
    x_slot = np.zeros((NCORES, NSHP, P), BF16)
    x_slot[ncore, lrow] = x.astype(BF16)

    key = ("prog", ep["T"], ep["NB"], ep["tb"].tobytes())
    if key not in _CACHE:
        _CACHE[key] = _build_program(ep)
    nc = _CACHE[key]

    in_maps = []
    for k in range(NCORES):
        m = dict(wd)
        m["xT"] = np.ascontiguousarray(x_slot[k].T)
        m["onehot"] = ep["onehot"][k]
        m["idxs"] = ep["idxs"][k]
        in_maps.append(m)

    res = run_bass_kernel_spmd(nc, in_maps, core_ids=list(range(NCORES)),
                               trace=trace)
    out_full = np.stack([res.results[k]["out"] for k in range(NCORES)])
    out = out_full[ncore, lrow].astype(np.float32)
    return out, res


def kernel(x, W1, att1, W2, att2, W3, att3, edge_index):
    out, _ = _run(dict(x=x, W1=W1, att1=att1, W2=W2, att2=att2, W3=W3,
                       att3=att3, edge_index=edge_index), trace=False)
    return out


def kernel_traced(inputs):
    return _run(inputs, trace=True)
